# revision 23
# baseline (speedup 1.0000x reference)
"""Trainium2 Bass kernel for a single-layer multi-head attention block.

Reference computation (per batch element):
    qkv = x @ w_qkv; q,k,v = split(qkv); 12 heads x 64
    out_h = softmax(q_h k_h^T / 8) v_h;  y = concat(out) @ w_out + b_out

Sharding: batch (8) data-parallel across 8 NeuronCores, one element/core.

Execution-backend model (measured with loop/unroll probes):
  - each *static* instruction costs ~50us on its first execution of a NEFF
    run (translation/fetch through the virtualization layer), while
    re-executing the same instructions inside a hardware loop costs only
    genuine engine time (~1-2us/matmul) plus ~55us per back-edge.
  - therefore the repeat body is emitted ONCE inside a tc.For_i hardware
    loop instead of being unrolled `reps` times: reps are identical
    computations into the same buffers, so the body needs no loop-var.
  - weight loads (w_qkv, w_out, bias, ones) are hoisted out of the loop and
    stay SBUF-resident; per-iteration work is x load + full compute +
    output store.

Numerics: all matmul operands are bf16 (PSUM accumulation stays fp32),
which quarters Tensor-engine time and halves SBUF/DMA traffic. Measured
rel err ~2e-3 against the fp32 reference, well inside the 2e-2 gate.
x is pre-transposed and pre-cast to bf16 on the host so the per-iteration
x load is a contiguous bf16 DMA.
"""

import time

import numpy as np
import ml_dtypes

import concourse.bacc as bacc
import concourse.mybir as mybir
import concourse.tile as tile
from concourse.bass_utils import run_bass_kernel_spmd

N_CORES = 8
N = 1024          # tokens per batch element
E = 768           # embedding dim
H = 12            # heads
D = 64            # head dim
P = 128

f32 = mybir.dt.float32
bf16 = mybir.dt.bfloat16
AF = mybir.ActivationFunctionType
BF16 = ml_dtypes.bfloat16


def build_nc(reps=1, abl=frozenset()):
    """abl: phase names to skip, for timing ablations only (suffix order:
    "store", "final", "av", "scores", "qkv")."""
    nc = bacc.Bacc("TRN2", target_bir_lowering=False, debug=False,
                   num_devices=N_CORES)

    xT_d = nc.dram_tensor("xT", [E, N], bf16, kind="ExternalInput")
    w_qkv = nc.dram_tensor("w_qkv", [E, 3 * E], bf16, kind="ExternalInput")
    w_out = nc.dram_tensor("w_out", [E, E], bf16, kind="ExternalInput")
    b_out = nc.dram_tensor("b_out", [E], f32, kind="ExternalInput")
    ones_c = nc.dram_tensor("ones_const", [1], bf16, kind="ExternalInput")
    out = nc.dram_tensor("out", [N, E], f32, kind="ExternalOutput")
    inv_scratch = nc.dram_tensor("inv_scratch", [H, N], f32)

    with tile.TileContext(nc) as tc:
      with tc.tile_pool(name="pers", bufs=1) as pers, \
           tc.tile_pool(name="ps1", bufs=1, space="PSUM") as ps1, \
           tc.tile_pool(name="ps2", bufs=1, space="PSUM") as ps2p, \
           tc.tile_pool(name="wq", bufs=1) as wq_pool, \
           tc.tile_pool(name="wout", bufs=1) as wout_pool:

        qkT = pers.tile([P, 12, N], bf16, tag="qkT")   # feat-major q|k
        v_aug = pers.tile([P, 8, H, D + 1], bf16, tag="v_aug")
        outT = pers.tile([P, 6, N], bf16, tag="outT")
        xT = pers.tile([P, 6, N], bf16, tag="xT")
        b_bc = pers.tile([P, E], f32, tag="b_bc")

        # ---- loop-invariant loads: weights, bias, ones column ----
        nc.sync.dma_start(out=b_bc[:], in_=b_out[None, :].to_broadcast((P, E)))
        nc.sync.dma_start(
            out=v_aug[:].rearrange("p a h d -> p (a h) d")[:, :, D:D + 1],
            in_=ones_c[None, None, :].to_broadcast((P, 8 * H, 1)))
        wqs = []
        for kc in range(6):
            wq = wq_pool.tile([P, 3 * E], bf16, tag=f"wq{kc}", name=f"wq_{kc}")
            nc.sync.dma_start(out=wq[:], in_=w_qkv[kc * P:(kc + 1) * P, :])
            wqs.append(wq)
        wos = []
        for fc in range(6):
            wo = wout_pool.tile([P, E], bf16, tag=f"wo{fc}", name=f"wo_{fc}")
            nc.sync.dma_start(out=wo[:], in_=w_out[fc * P:(fc + 1) * P, :])
            wos.append(wo)

        dma_engines = (nc.sync, nc.scalar, nc.gpsimd)

        with tc.For_i(0, reps, 1, staggered_reset=True,
                      hint_engines=tuple(mybir.ALL_ENGINES)):
            # ---- x load: contiguous bf16 (pre-transposed on host),
            #      spread across DMA queues ----
            for ec in range(6):
                dma_engines[ec % 3].dma_start(
                    out=xT[:, ec, :], in_=xT_d[ec * P:(ec + 1) * P, :])

            # ---- phase 1: qkT (feat-major) and v (token-major, augmented) ----
            for jg in (range(4) if "qkv" not in abl else ()):  # groups of 3 j's
                pq = ps2p.tile([P, 3 * N], f32, tag="ps2", name=f"pq_{jg}")
                for sj in range(3):
                    j = jg * 3 + sj
                    for kc in range(6):       # kc outer: nt pair shares lhsT
                        for nt in range(2):
                            nc.tensor.matmul(
                                pq[:, sj * N + nt * 512:
                                   sj * N + (nt + 1) * 512],
                                wqs[kc][:, j * P:(j + 1) * P],
                                xT[:, kc, nt * 512:(nt + 1) * 512],
                                start=(kc == 0), stop=(kc == 5))
                nc.vector.tensor_copy(qkT[:, jg * 3:(jg + 1) * 3, :], pq[:])

            for tg, gt in (((0, 3), (3, 3), (6, 2))
                           if "qkv" not in abl else ()):  # groups of 3|2 t's
                pv = ps2p.tile([P, 3 * N], f32, tag="ps2", name=f"pv_{tg}")
                for st in range(gt):
                    t = tg + st
                    for kc in range(6):       # kc outer: vf pair shares lhsT
                        for vf, f0, fw in ((0, 0, 512), (1, 512, 256)):
                            nc.tensor.matmul(
                                pv[:, st * N + f0:st * N + f0 + fw],
                                xT[:, kc, t * P:(t + 1) * P],
                                wqs[kc][:, 2 * E + f0:2 * E + f0 + fw],
                                start=(kc == 0), stop=(kc == 5))
                nc.vector.tensor_copy(
                    v_aug[:, tg:tg + gt, :, 0:D],
                    pv[:].rearrange("p (t e) -> p t e", e=N)[:, 0:gt, 0:E]
                    .rearrange("p t (h d) -> p t h d", d=D))

            # staggered-reset stage boundaries pinned to phase edges so the
            # 4 stages are [xload+qkv | heads 0-5 | heads 6-11 | final]
            # instead of equal splits that cut mid-pipeline
            tc.stage_boundary()

            # ---- attention per head ----
            with tc.tile_pool(name="expp", bufs=3) as exp_pool, \
                 tc.tile_pool(name="scp", bufs=2) as sc_pool, \
                 tc.tile_pool(name="invp", bufs=2) as inv_pool, \
                 tc.tile_pool(name="ibcp", bufs=2) as ibc_pool:
                # key-tile groups per exp: 3+3+2 key tiles -> 3 ACTIVATEs/head
                m_groups = ((0, 3), (3, 3), (6, 2))
                for h in (range(H) if "scores" not in abl else ()):
                    qp = (h % 2) * D
                    jq = h // 2
                    jk = 6 + h // 2
                    exps = []
                    for gi, (m0, gm) in enumerate(m_groups):
                        ps2 = ps2p.tile([P, 3 * N], f32, tag="ps2",
                                        name=f"ps2_{h}_{gi}")
                        for s2 in range(gm):
                            m = m0 + s2
                            for nt in range(2):
                                nc.tensor.matmul(
                                    ps2[:, s2 * N + nt * 512:
                                        s2 * N + (nt + 1) * 512],
                                    qkT[qp:qp + D, jk, m * P:(m + 1) * P],
                                    qkT[qp:qp + D, jq, nt * 512:(nt + 1) * 512],
                                    start=True, stop=True)
                        # cast scores to bf16 on DVE first: 16-bit input
                        # doubles ACT exp throughput, and DVE is mostly idle
                        sc16 = sc_pool.tile([P, 3 * N], bf16, tag="scp",
                                            name=f"sc16_{h}_{gi}")
                        nc.vector.tensor_copy(sc16[:, 0:gm * N],
                                              ps2[:, 0:gm * N])
                        et = exp_pool.tile([P, 3 * N], bf16, tag="expp",
                                           name=f"exp_{h}_{gi}")
                        nc.scalar.activation(et[:, 0:gm * N], sc16[:, 0:gm * N],
                                             AF.Exp, scale=0.125)
                        exps.append(et)
                    if "av" in abl:
                        continue
                    pav = ps1.tile([P, N], f32, tag="ps1", name=f"pav_{h}")
                    for kc in range(8):
                        gi = min(kc // 3, 2)
                        off = (kc - m_groups[gi][0]) * N
                        for nt in range(2):
                            nc.tensor.matmul(
                                pav[0:D + 1, nt * 512:(nt + 1) * 512],
                                v_aug[:, kc, h, :],
                                exps[gi][:, off + nt * 512:off + (nt + 1) * 512],
                                start=(kc == 0), stop=(kc == 7))
                    inv = inv_pool.tile([D + 1, N], f32, tag="invp",
                                        name=f"inv_{h}")
                    nc.vector.reciprocal(inv[D:D + 1, :], pav[D:D + 1, :])
                    ibc = ibc_pool.tile([D, N], f32, tag="ibcp",
                                        name=f"ibc_{h}")
                    # partition-broadcast via DRAM bounce (SBUF sources must
                    # have nonzero partition step, DRAM reads may broadcast)
                    dma_engines[h % 3].dma_start(
                        out=inv_scratch[h][None, :], in_=inv[D:D + 1, :])
                    dma_engines[(h + 1) % 3].dma_start(
                        out=ibc[:],
                        in_=inv_scratch[h][None, :].to_broadcast((D, N)))
                    nc.vector.tensor_mul(outT[qp:qp + D, jq, :],
                                         pav[0:D, :], ibc[:])
                    if h == 5:
                        tc.stage_boundary()

            tc.stage_boundary()

            # ---- output projection + bias ----
            with tc.tile_pool(name="finp", bufs=1) as fin_pool:
                fstage = fin_pool.tile([P, 8, E], f32, tag="fin",
                                       name="fstage")
                for t in (range(8) if "final" not in abl else ()):
                    pf = ps1.tile([P, N], f32, tag="ps1", name=f"pf_{t}")
                    for fc in range(6):       # fc outer: ft pair shares lhsT
                        for ft, f0, fw in ((0, 0, 512), (1, 512, 256)):
                            nc.tensor.matmul(
                                pf[:, f0:f0 + fw],
                                outT[:, fc, t * P:(t + 1) * P],
                                wos[fc][:, f0:f0 + fw],
                                start=(fc == 0), stop=(fc == 5))
                    nc.vector.tensor_add(fstage[:, t, :], pf[:, 0:E], b_bc[:])
                if "final" not in abl and "store" not in abl:
                    out_v = out.rearrange("(t p) e -> p t e", p=P)
                    for sq in range(4):   # split the 3MB store over queues
                        dma_engines[sq % 3].dma_start(
                            out=out_v[:, 2 * sq:2 * sq + 2, :],
                            in_=fstage[:, 2 * sq:2 * sq + 2, :])
                else:
                    nc.sync.dma_start(out=out[0:P, :], in_=b_bc[:])

    nc.compile()
    return nc


_NC = None


def _get_nc():
    global _NC
    if _NC is None:
        _NC = build_nc()
    return _NC


def make_in_maps(x, w_qkv, w_out, b_out):
    """Host-side input marshalling: per-core transposed bf16 x + shared
    bf16 weights."""
    x = np.asarray(x)
    wq16 = np.ascontiguousarray(np.asarray(w_qkv, dtype=np.float32)
                                .astype(BF16))
    wo16 = np.ascontiguousarray(np.asarray(w_out, dtype=np.float32)
                                .astype(BF16))
    b_out = np.ascontiguousarray(np.asarray(b_out, dtype=np.float32))
    one = np.ones(1, dtype=BF16)
    return [
        {"xT": np.ascontiguousarray(
             np.asarray(x[i], dtype=np.float32).T.astype(BF16)),
         "w_qkv": wq16, "w_out": wo16, "b_out": b_out, "ones_const": one}
        for i in range(N_CORES)
    ]


def kernel(x, w_qkv, w_out, b_out):
    nc = _get_nc()
    in_maps = make_in_maps(x, w_qkv, w_out, b_out)
    last_exc = None
    for attempt in range(4):   # retry transient device errors
        try:
            res = run_bass_kernel_spmd(nc, in_maps,
                                       core_ids=list(range(N_CORES)))
            return np.stack([res.results[i]["out"] for i in range(N_CORES)],
                            axis=0)
        except Exception as e:   # noqa: BLE001
            last_exc = e
            time.sleep(2.0 * (attempt + 1))
    raise last_exc


# revision 24
# speedup vs baseline: 1.1765x; 1.1765x over previous
"""Trainium2 Bass kernel for a single-layer multi-head attention block.

Reference computation (per batch element):
    qkv = x @ w_qkv; q,k,v = split(qkv); 12 heads x 64
    out_h = softmax(q_h k_h^T / 8) v_h;  y = concat(out) @ w_out + b_out

Sharding: batch (8) data-parallel across 8 NeuronCores, one element/core.

Execution-backend model (measured with loop/unroll probes):
  - each *static* instruction costs ~50us on its first execution of a NEFF
    run (translation/fetch through the virtualization layer), while
    re-executing the same instructions inside a hardware loop costs only
    genuine engine time (~1-2us/matmul) plus ~55us per back-edge.
  - therefore the repeat body is emitted ONCE inside a tc.For_i hardware
    loop instead of being unrolled `reps` times: reps are identical
    computations into the same buffers, so the body needs no loop-var.
  - weight loads (w_qkv, w_out, bias, ones) are hoisted out of the loop and
    stay SBUF-resident; per-iteration work is x load + full compute +
    output store.

Numerics: all matmul operands are bf16 (PSUM accumulation stays fp32),
which quarters Tensor-engine time and halves SBUF/DMA traffic. Measured
rel err ~2e-3 against the fp32 reference, well inside the 2e-2 gate.
x is pre-transposed and pre-cast to bf16 on the host so the per-iteration
x load is a contiguous bf16 DMA.
"""

import time

import numpy as np
import ml_dtypes

import concourse.bacc as bacc
import concourse.mybir as mybir
import concourse.tile as tile
from concourse.bass_utils import run_bass_kernel_spmd

N_CORES = 8
N = 1024          # tokens per batch element
E = 768           # embedding dim
H = 12            # heads
D = 64            # head dim
P = 128

f32 = mybir.dt.float32
bf16 = mybir.dt.bfloat16
AF = mybir.ActivationFunctionType
BF16 = ml_dtypes.bfloat16


def build_nc(reps=1, abl=frozenset()):
    """abl: phase names to skip, for timing ablations only (suffix order:
    "store", "final", "av", "scores", "qkv")."""
    nc = bacc.Bacc("TRN2", target_bir_lowering=False, debug=False,
                   num_devices=N_CORES)

    xT_d = nc.dram_tensor("xT", [E, N], bf16, kind="ExternalInput")
    w_qkv = nc.dram_tensor("w_qkv", [E, 3 * E], bf16, kind="ExternalInput")
    w_out = nc.dram_tensor("w_out", [E, E], bf16, kind="ExternalInput")
    b_out = nc.dram_tensor("b_out", [E], f32, kind="ExternalInput")
    ones_c = nc.dram_tensor("ones_const", [1], bf16, kind="ExternalInput")
    out = nc.dram_tensor("out", [N, E], f32, kind="ExternalOutput")
    inv_scratch = nc.dram_tensor("inv_scratch", [H, N], f32)

    with tile.TileContext(nc) as tc:
      with tc.tile_pool(name="pers", bufs=1) as pers, \
           tc.tile_pool(name="ps1", bufs=1, space="PSUM") as ps1, \
           tc.tile_pool(name="ps2", bufs=1, space="PSUM") as ps2p, \
           tc.tile_pool(name="wq", bufs=1) as wq_pool, \
           tc.tile_pool(name="wout", bufs=1) as wout_pool:

        qkT = pers.tile([P, 12, N], bf16, tag="qkT")   # feat-major q|k
        v_aug = pers.tile([P, 8, H, D + 1], bf16, tag="v_aug")
        outT = pers.tile([P, 6, N], bf16, tag="outT")
        xT = pers.tile([P, 6, N], bf16, tag="xT")
        b_bc = pers.tile([P, E], f32, tag="b_bc")

        # ---- loop-invariant loads: weights, bias, ones column ----
        nc.sync.dma_start(out=b_bc[:], in_=b_out[None, :].to_broadcast((P, E)))
        nc.sync.dma_start(
            out=v_aug[:].rearrange("p a h d -> p (a h) d")[:, :, D:D + 1],
            in_=ones_c[None, None, :].to_broadcast((P, 8 * H, 1)))
        wqs = []
        for kc in range(6):
            wq = wq_pool.tile([P, 3 * E], bf16, tag=f"wq{kc}", name=f"wq_{kc}")
            nc.sync.dma_start(out=wq[:], in_=w_qkv[kc * P:(kc + 1) * P, :])
            wqs.append(wq)
        wos = []
        for fc in range(6):
            wo = wout_pool.tile([P, E], bf16, tag=f"wo{fc}", name=f"wo_{fc}")
            nc.sync.dma_start(out=wo[:], in_=w_out[fc * P:(fc + 1) * P, :])
            wos.append(wo)

        dma_engines = (nc.sync, nc.scalar, nc.gpsimd)

        with tc.For_i(0, reps, 1, staggered_reset=True,
                      hint_engines=tuple(mybir.ALL_ENGINES)):
            # ---- x load: contiguous bf16 (pre-transposed on host),
            #      spread across DMA queues ----
            for ec in range(6):
                dma_engines[ec % 3].dma_start(
                    out=xT[:, ec, :], in_=xT_d[ec * P:(ec + 1) * P, :])

            # ---- phase 1: qkT (feat-major) and v (token-major, augmented) ----
            for jg in (range(4) if "qkv" not in abl else ()):  # groups of 3 j's
                pq = ps2p.tile([P, 3 * N], f32, tag="ps2", name=f"pq_{jg}")
                for sj in range(3):
                    j = jg * 3 + sj
                    for kc in range(6):       # kc outer: nt pair shares lhsT
                        for nt in range(2):
                            nc.tensor.matmul(
                                pq[:, sj * N + nt * 512:
                                   sj * N + (nt + 1) * 512],
                                wqs[kc][:, j * P:(j + 1) * P],
                                xT[:, kc, nt * 512:(nt + 1) * 512],
                                start=(kc == 0), stop=(kc == 5))
                nc.vector.tensor_copy(qkT[:, jg * 3:(jg + 1) * 3, :], pq[:])

            for tg, gt in (((0, 3), (3, 3), (6, 2))
                           if "qkv" not in abl else ()):  # groups of 3|2 t's
                pv = ps2p.tile([P, 3 * N], f32, tag="ps2", name=f"pv_{tg}")
                for st in range(gt):
                    t = tg + st
                    for kc in range(6):       # kc outer: vf pair shares lhsT
                        for vf, f0, fw in ((0, 0, 512), (1, 512, 256)):
                            nc.tensor.matmul(
                                pv[:, st * N + f0:st * N + f0 + fw],
                                xT[:, kc, t * P:(t + 1) * P],
                                wqs[kc][:, 2 * E + f0:2 * E + f0 + fw],
                                start=(kc == 0), stop=(kc == 5))
                nc.vector.tensor_copy(
                    v_aug[:, tg:tg + gt, :, 0:D],
                    pv[:].rearrange("p (t e) -> p t e", e=N)[:, 0:gt, 0:E]
                    .rearrange("p t (h d) -> p t h d", d=D))

            # ---- attention per head ----
            with tc.tile_pool(name="expp", bufs=3) as exp_pool, \
                 tc.tile_pool(name="scp", bufs=2) as sc_pool, \
                 tc.tile_pool(name="invp", bufs=2) as inv_pool, \
                 tc.tile_pool(name="ibcp", bufs=2) as ibc_pool:
                # key-tile groups per exp: 3+3+2 key tiles -> 3 ACTIVATEs/head
                m_groups = ((0, 3), (3, 3), (6, 2))
                for h in (range(H) if "scores" not in abl else ()):
                    qp = (h % 2) * D
                    jq = h // 2
                    jk = 6 + h // 2
                    exps = []
                    for gi, (m0, gm) in enumerate(m_groups):
                        ps2 = ps2p.tile([P, 3 * N], f32, tag="ps2",
                                        name=f"ps2_{h}_{gi}")
                        for s2 in range(gm):
                            m = m0 + s2
                            for nt in range(2):
                                nc.tensor.matmul(
                                    ps2[:, s2 * N + nt * 512:
                                        s2 * N + (nt + 1) * 512],
                                    qkT[qp:qp + D, jk, m * P:(m + 1) * P],
                                    qkT[qp:qp + D, jq, nt * 512:(nt + 1) * 512],
                                    start=True, stop=True)
                        # cast scores to bf16 on DVE first: 16-bit input
                        # doubles ACT exp throughput, and DVE is mostly idle
                        sc16 = sc_pool.tile([P, 3 * N], bf16, tag="scp",
                                            name=f"sc16_{h}_{gi}")
                        nc.vector.tensor_copy(sc16[:, 0:gm * N],
                                              ps2[:, 0:gm * N])
                        et = exp_pool.tile([P, 3 * N], bf16, tag="expp",
                                           name=f"exp_{h}_{gi}")
                        nc.scalar.activation(et[:, 0:gm * N], sc16[:, 0:gm * N],
                                             AF.Exp, scale=0.125)
                        exps.append(et)
                    if "av" in abl:
                        continue
                    pav = ps1.tile([P, N], f32, tag="ps1", name=f"pav_{h}")
                    for kc in range(8):
                        gi = min(kc // 3, 2)
                        off = (kc - m_groups[gi][0]) * N
                        for nt in range(2):
                            nc.tensor.matmul(
                                pav[0:D + 1, nt * 512:(nt + 1) * 512],
                                v_aug[:, kc, h, :],
                                exps[gi][:, off + nt * 512:off + (nt + 1) * 512],
                                start=(kc == 0), stop=(kc == 7))
                    inv = inv_pool.tile([D + 1, N], f32, tag="invp",
                                        name=f"inv_{h}")
                    nc.vector.reciprocal(inv[D:D + 1, :], pav[D:D + 1, :])
                    ibc = ibc_pool.tile([D, N], f32, tag="ibcp",
                                        name=f"ibc_{h}")
                    # partition-broadcast via DRAM bounce (SBUF sources must
                    # have nonzero partition step, DRAM reads may broadcast)
                    dma_engines[h % 3].dma_start(
                        out=inv_scratch[h][None, :], in_=inv[D:D + 1, :])
                    dma_engines[(h + 1) % 3].dma_start(
                        out=ibc[:],
                        in_=inv_scratch[h][None, :].to_broadcast((D, N)))
                    nc.vector.tensor_mul(outT[qp:qp + D, jq, :],
                                         pav[0:D, :], ibc[:])

            # ---- output projection + bias ----
            with tc.tile_pool(name="finp", bufs=1) as fin_pool:
                fstage = fin_pool.tile([P, 8, E], f32, tag="fin",
                                       name="fstage")
                for t in (range(8) if "final" not in abl else ()):
                    pf = ps1.tile([P, N], f32, tag="ps1", name=f"pf_{t}")
                    for fc in range(6):       # fc outer: ft pair shares lhsT
                        for ft, f0, fw in ((0, 0, 512), (1, 512, 256)):
                            nc.tensor.matmul(
                                pf[:, f0:f0 + fw],
                                outT[:, fc, t * P:(t + 1) * P],
                                wos[fc][:, f0:f0 + fw],
                                start=(fc == 0), stop=(fc == 5))
                    nc.vector.tensor_add(fstage[:, t, :], pf[:, 0:E], b_bc[:])
                if "final" not in abl and "store" not in abl:
                    out_v = out.rearrange("(t p) e -> p t e", p=P)
                    for sq in range(4):   # split the 3MB store over queues
                        dma_engines[sq % 3].dma_start(
                            out=out_v[:, 2 * sq:2 * sq + 2, :],
                            in_=fstage[:, 2 * sq:2 * sq + 2, :])
                else:
                    nc.sync.dma_start(out=out[0:P, :], in_=b_bc[:])

    nc.compile()
    return nc


_NC = None


def _get_nc():
    global _NC
    if _NC is None:
        _NC = build_nc()
    return _NC


def make_in_maps(x, w_qkv, w_out, b_out):
    """Host-side input marshalling: per-core transposed bf16 x + shared
    bf16 weights."""
    x = np.asarray(x)
    wq16 = np.ascontiguousarray(np.asarray(w_qkv, dtype=np.float32)
                                .astype(BF16))
    wo16 = np.ascontiguousarray(np.asarray(w_out, dtype=np.float32)
                                .astype(BF16))
    b_out = np.ascontiguousarray(np.asarray(b_out, dtype=np.float32))
    one = np.ones(1, dtype=BF16)
    return [
        {"xT": np.ascontiguousarray(
             np.asarray(x[i], dtype=np.float32).T.astype(BF16)),
         "w_qkv": wq16, "w_out": wo16, "b_out": b_out, "ones_const": one}
        for i in range(N_CORES)
    ]


def kernel(x, w_qkv, w_out, b_out):
    nc = _get_nc()
    in_maps = make_in_maps(x, w_qkv, w_out, b_out)
    last_exc = None
    for attempt in range(4):   # retry transient device errors
        try:
            res = run_bass_kernel_spmd(nc, in_maps,
                                       core_ids=list(range(N_CORES)))
            return np.stack([res.results[i]["out"] for i in range(N_CORES)],
                            axis=0)
        except Exception as e:   # noqa: BLE001
            last_exc = e
            time.sleep(2.0 * (attempt + 1))
    raise last_exc


# revision 27
# speedup vs baseline: 1.2124x; 1.0305x over previous
"""Trainium2 Bass kernel for a single-layer multi-head attention block.

Reference computation (per batch element):
    qkv = x @ w_qkv; q,k,v = split(qkv); 12 heads x 64
    out_h = softmax(q_h k_h^T / 8) v_h;  y = concat(out) @ w_out + b_out

Sharding: batch (8) data-parallel across 8 NeuronCores, one element/core.

Execution-backend model (measured with loop/unroll probes):
  - each *static* instruction costs ~50us on its first execution of a NEFF
    run (translation/fetch through the virtualization layer), while
    re-executing the same instructions inside a hardware loop costs only
    genuine engine time (~1-2us/matmul) plus ~55us per back-edge.
  - therefore the repeat body is emitted ONCE inside a tc.For_i hardware
    loop instead of being unrolled `reps` times: reps are identical
    computations into the same buffers, so the body needs no loop-var.
  - weight loads (w_qkv, w_out, bias, ones) are hoisted out of the loop and
    stay SBUF-resident; per-iteration work is x load + full compute +
    output store.

Numerics: all matmul operands are bf16 (PSUM accumulation stays fp32),
which quarters Tensor-engine time and halves SBUF/DMA traffic. Measured
rel err ~2e-3 against the fp32 reference, well inside the 2e-2 gate.
x is pre-transposed and pre-cast to bf16 on the host so the per-iteration
x load is a contiguous bf16 DMA.
"""

import time

import numpy as np
import ml_dtypes

import concourse.bacc as bacc
import concourse.mybir as mybir
import concourse.tile as tile
from concourse.bass_utils import run_bass_kernel_spmd

N_CORES = 8
N = 1024          # tokens per batch element
E = 768           # embedding dim
H = 12            # heads
D = 64            # head dim
P = 128

f32 = mybir.dt.float32
bf16 = mybir.dt.bfloat16
AF = mybir.ActivationFunctionType
BF16 = ml_dtypes.bfloat16

# heads whose scores+exp are emitted before the v projection, giving the
# Scalar engine a head start on the exp pipeline while the PE is still
# busy with the v matmuls
EARLY_HEADS = 2


def build_nc(reps=1, abl=frozenset()):
    """abl: phase names to skip, for timing ablations only (suffix order:
    "store", "final", "av", "scores", "qkv")."""
    nc = bacc.Bacc("TRN2", target_bir_lowering=False, debug=False,
                   num_devices=N_CORES)

    xT_d = nc.dram_tensor("xT", [E, N], bf16, kind="ExternalInput")
    w_qkv = nc.dram_tensor("w_qkv", [E, 3 * E], bf16, kind="ExternalInput")
    w_out = nc.dram_tensor("w_out", [E, E], bf16, kind="ExternalInput")
    b_out = nc.dram_tensor("b_out", [E], f32, kind="ExternalInput")
    ones_c = nc.dram_tensor("ones_const", [1], bf16, kind="ExternalInput")
    out = nc.dram_tensor("out", [N, E], f32, kind="ExternalOutput")
    inv_scratch = nc.dram_tensor("inv_scratch", [H, N], f32)

    with tile.TileContext(nc) as tc:
      with tc.tile_pool(name="pers", bufs=1) as pers, \
           tc.tile_pool(name="ps1", bufs=1, space="PSUM") as ps1, \
           tc.tile_pool(name="ps2", bufs=1, space="PSUM") as ps2p, \
           tc.tile_pool(name="wq", bufs=1) as wq_pool, \
           tc.tile_pool(name="wout", bufs=1) as wout_pool:

        qkT = pers.tile([P, 12, N], bf16, tag="qkT")   # feat-major q|k
        v_aug = pers.tile([P, 8, H, D + 1], bf16, tag="v_aug")
        outT = pers.tile([P, 6, N], bf16, tag="outT")
        xT = pers.tile([P, 6, N], bf16, tag="xT")
        b_bc = pers.tile([P, E], f32, tag="b_bc")

        # ---- loop-invariant loads: weights, bias, ones column ----
        nc.sync.dma_start(out=b_bc[:], in_=b_out[None, :].to_broadcast((P, E)))
        nc.sync.dma_start(
            out=v_aug[:].rearrange("p a h d -> p (a h) d")[:, :, D:D + 1],
            in_=ones_c[None, None, :].to_broadcast((P, 8 * H, 1)))
        wqs = []
        for kc in range(6):
            wq = wq_pool.tile([P, 3 * E], bf16, tag=f"wq{kc}", name=f"wq_{kc}")
            nc.sync.dma_start(out=wq[:], in_=w_qkv[kc * P:(kc + 1) * P, :])
            wqs.append(wq)
        wos = []
        for fc in range(6):
            wo = wout_pool.tile([P, E], bf16, tag=f"wo{fc}", name=f"wo_{fc}")
            nc.sync.dma_start(out=wo[:], in_=w_out[fc * P:(fc + 1) * P, :])
            wos.append(wo)

        dma_engines = (nc.sync, nc.scalar, nc.gpsimd)

        with tc.For_i(0, reps, 1, staggered_reset=True,
                      hint_engines=tuple(mybir.ALL_ENGINES)):
            # ---- x load: contiguous bf16 (pre-transposed on host),
            #      spread across DMA queues ----
            for ec in range(6):
                dma_engines[ec % 3].dma_start(
                    out=xT[:, ec, :], in_=xT_d[ec * P:(ec + 1) * P, :])

            # ---- phase 1 + attention ----
            with tc.tile_pool(name="expp", bufs=3 + 3 * EARLY_HEADS) \
                    as exp_pool, \
                 tc.tile_pool(name="scp", bufs=3) as sc_pool, \
                 tc.tile_pool(name="invp", bufs=2) as inv_pool, \
                 tc.tile_pool(name="ibcp", bufs=2) as ibc_pool:
                # key-tile groups per exp: 3+3+2 key tiles -> 3 ACTIVATEs/head
                m_groups = ((0, 3), (3, 3), (6, 2))

                def emit_scores_exp(h):
                    qp = (h % 2) * D
                    jq = h // 2
                    jk = 6 + h // 2
                    exps = []
                    for gi, (m0, gm) in enumerate(m_groups):
                        ps2 = ps2p.tile([P, 3 * N], f32, tag="ps2",
                                        name=f"ps2_{h}_{gi}")
                        for s2 in range(gm):
                            m = m0 + s2
                            for nt in range(2):
                                nc.tensor.matmul(
                                    ps2[:, s2 * N + nt * 512:
                                        s2 * N + (nt + 1) * 512],
                                    qkT[qp:qp + D, jk, m * P:(m + 1) * P],
                                    qkT[qp:qp + D, jq, nt * 512:(nt + 1) * 512],
                                    start=True, stop=True)
                        # cast scores to bf16 on DVE first: 16-bit input
                        # doubles ACT exp throughput, and DVE is mostly idle
                        sc16 = sc_pool.tile([P, 3 * N], bf16, tag="scp",
                                            name=f"sc16_{h}_{gi}")
                        nc.vector.tensor_copy(sc16[:, 0:gm * N],
                                              ps2[:, 0:gm * N])
                        et = exp_pool.tile([P, 3 * N], bf16, tag="expp",
                                           name=f"exp_{h}_{gi}")
                        nc.scalar.activation(et[:, 0:gm * N],
                                             sc16[:, 0:gm * N],
                                             AF.Exp, scale=0.125)
                        exps.append(et)
                    return exps

                for jg in (range(4) if "qkv" not in abl else ()):
                    pq = ps2p.tile([P, 3 * N], f32, tag="ps2",
                                   name=f"pq_{jg}")
                    for sj in range(3):
                        j = jg * 3 + sj
                        for kc in range(6):   # kc outer: nt pair shares lhsT
                            for nt in range(2):
                                nc.tensor.matmul(
                                    pq[:, sj * N + nt * 512:
                                       sj * N + (nt + 1) * 512],
                                    wqs[kc][:, j * P:(j + 1) * P],
                                    xT[:, kc, nt * 512:(nt + 1) * 512],
                                    start=(kc == 0), stop=(kc == 5))
                    nc.vector.tensor_copy(qkT[:, jg * 3:(jg + 1) * 3, :],
                                          pq[:])

                # early heads: scores+exp emitted before the v projection so
                # the ACT exp pipeline overlaps the v matmuls
                early = {}
                if "scores" not in abl and "qkv" not in abl:
                    for h in range(EARLY_HEADS):
                        early[h] = emit_scores_exp(h)

                for tg, gt in (((0, 3), (3, 3), (6, 2))
                               if "qkv" not in abl else ()):
                    pv = ps2p.tile([P, 3 * N], f32, tag="ps2",
                                   name=f"pv_{tg}")
                    for st in range(gt):
                        t = tg + st
                        for kc in range(6):   # kc outer: vf pair shares lhsT
                            for vf, f0, fw in ((0, 0, 512), (1, 512, 256)):
                                nc.tensor.matmul(
                                    pv[:, st * N + f0:st * N + f0 + fw],
                                    xT[:, kc, t * P:(t + 1) * P],
                                    wqs[kc][:, 2 * E + f0:2 * E + f0 + fw],
                                    start=(kc == 0), stop=(kc == 5))
                    nc.vector.tensor_copy(
                        v_aug[:, tg:tg + gt, :, 0:D],
                        pv[:].rearrange("p (t e) -> p t e", e=N)[:, 0:gt, 0:E]
                        .rearrange("p t (h d) -> p t h d", d=D))

                # ---- attention per head ----
                for h in (range(H) if "scores" not in abl else ()):
                    qp = (h % 2) * D
                    jq = h // 2
                    exps = early[h] if h in early else emit_scores_exp(h)
                    if "av" in abl:
                        continue
                    pav = ps1.tile([P, N], f32, tag="ps1", name=f"pav_{h}")
                    for kc in range(8):
                        gi = min(kc // 3, 2)
                        off = (kc - m_groups[gi][0]) * N
                        for nt in range(2):
                            nc.tensor.matmul(
                                pav[0:D + 1, nt * 512:(nt + 1) * 512],
                                v_aug[:, kc, h, :],
                                exps[gi][:, off + nt * 512:off + (nt + 1) * 512],
                                start=(kc == 0), stop=(kc == 7))
                    inv = inv_pool.tile([D + 1, N], f32, tag="invp",
                                        name=f"inv_{h}")
                    nc.vector.reciprocal(inv[D:D + 1, :], pav[D:D + 1, :])
                    ibc = ibc_pool.tile([D, N], f32, tag="ibcp",
                                        name=f"ibc_{h}")
                    # partition-broadcast via DRAM bounce (SBUF sources must
                    # have nonzero partition step, DRAM reads may broadcast)
                    dma_engines[h % 3].dma_start(
                        out=inv_scratch[h][None, :], in_=inv[D:D + 1, :])
                    dma_engines[(h + 1) % 3].dma_start(
                        out=ibc[:],
                        in_=inv_scratch[h][None, :].to_broadcast((D, N)))
                    nc.vector.tensor_mul(outT[qp:qp + D, jq, :],
                                         pav[0:D, :], ibc[:])

            # ---- output projection + bias ----
            with tc.tile_pool(name="finp", bufs=1) as fin_pool:
                fstage = fin_pool.tile([P, 8, E], f32, tag="fin",
                                       name="fstage")
                out_v = out.rearrange("(t p) e -> p t e", p=P)
                for t in (range(8) if "final" not in abl else ()):
                    pf = ps1.tile([P, N], f32, tag="ps1", name=f"pf_{t}")
                    for fc in range(6):       # fc outer: ft pair shares lhsT
                        for ft, f0, fw in ((0, 0, 512), (1, 512, 256)):
                            nc.tensor.matmul(
                                pf[:, f0:f0 + fw],
                                outT[:, fc, t * P:(t + 1) * P],
                                wos[fc][:, f0:f0 + fw],
                                start=(fc == 0), stop=(fc == 5))
                    nc.vector.tensor_add(fstage[:, t, :], pf[:, 0:E], b_bc[:])
                    if "store" not in abl:
                        # store each token-tile as soon as its bias add
                        # lands, draining the 3MB store during the proj
                        dma_engines[t % 3].dma_start(
                            out=out_v[:, t, :], in_=fstage[:, t, :])
                if "final" in abl or "store" in abl:
                    nc.sync.dma_start(out=out[0:P, :], in_=b_bc[:])

    nc.compile()
    return nc


_NC = None


def _get_nc():
    global _NC
    if _NC is None:
        _NC = build_nc()
    return _NC


def make_in_maps(x, w_qkv, w_out, b_out):
    """Host-side input marshalling: per-core transposed bf16 x + shared
    bf16 weights."""
    x = np.asarray(x)
    wq16 = np.ascontiguousarray(np.asarray(w_qkv, dtype=np.float32)
                                .astype(BF16))
    wo16 = np.ascontiguousarray(np.asarray(w_out, dtype=np.float32)
                                .astype(BF16))
    b_out = np.ascontiguousarray(np.asarray(b_out, dtype=np.float32))
    one = np.ones(1, dtype=BF16)
    return [
        {"xT": np.ascontiguousarray(
             np.asarray(x[i], dtype=np.float32).T.astype(BF16)),
         "w_qkv": wq16, "w_out": wo16, "b_out": b_out, "ones_const": one}
        for i in range(N_CORES)
    ]


def kernel(x, w_qkv, w_out, b_out):
    nc = _get_nc()
    in_maps = make_in_maps(x, w_qkv, w_out, b_out)
    last_exc = None
    for attempt in range(4):   # retry transient device errors
        try:
            res = run_bass_kernel_spmd(nc, in_maps,
                                       core_ids=list(range(N_CORES)))
            return np.stack([res.results[i]["out"] for i in range(N_CORES)],
                            axis=0)
        except Exception as e:   # noqa: BLE001
            last_exc = e
            time.sleep(2.0 * (attempt + 1))
    raise last_exc
